# revision 1
# baseline (speedup 1.0000x reference)
"""MLA attention (DeepSeek-style) distributed over 8 TRN2 NeuronCores.

Sharding: core c -> batch b=c//4, head-group/seq-shard g=c%4.
Phase A: down-proj of own 512-pos shard -> bf16 bounce -> 8-core AllGathers
(kv first, then q split in two, so comms overlap projection compute).
Phase B: up-proj (Qt/Kt d-major, V row-major), flash-style causal attention
with St[kv,q] layout (softmax denominators via ones-matmul, no transposes).
Attention outputs AllGather per head; the row-parallel wo matmul on the own
s-shard is interleaved between attention heads and accumulated in SBUF.
"""

import numpy as np
import ml_dtypes

import concourse.bass as bass
import concourse.bacc as bacc
import concourse.tile as tile
import concourse.mybir as mybir
from concourse.bass_utils import run_bass_kernel_spmd

BF16 = ml_dtypes.bfloat16

# problem constants (hardcoded per harness rules)
DIM = 2048
N_HEADS = 16
Q_LORA = 1536
KV_LORA = 512
NOPE = 128
ROPE = 64
V_DIM = 128
QK_HD = NOPE + ROPE  # 192
EPS = 1e-6
B, S = 2, 2048
SCALE = QK_HD ** -0.5

NCORES = 8
GROUP = 4               # cores per batch
SSH = S // GROUP        # 512, seq shard
HPC = N_HEADS // GROUP  # 4 heads per core
P = 128
NKT = DIM // P          # 16
NQM = Q_LORA // P       # 12
NQ1 = 6                 # q slabs in first AG
NKVM = KV_LORA // P     # 4
NCH = S // 512          # 4
KVROWS = KV_LORA + ROPE + 1   # 577 kv bounce rows
Q1ROWS = NQ1 * P              # 768
Q2ROWS = (NQM - NQ1) * P + 1  # 769 (incl a_q row)

_cache = {}


def _build():
    nc = bacc.Bacc("TRN2", target_bir_lowering=False, debug=False,
                   num_devices=NCORES)
    f32 = mybir.dt.float32
    bf = mybir.dt.bfloat16
    i32 = mybir.dt.int32

    # ---- dram parameters ----
    xT = nc.dram_tensor("xT", [DIM, SSH], bf, kind="ExternalInput")
    wqaT = nc.dram_tensor("wqaT", [DIM, Q_LORA], bf, kind="ExternalInput")
    wkvaT = nc.dram_tensor("wkvaT", [DIM, KV_LORA + ROPE], bf,
                           kind="ExternalInput")
    wqbT = nc.dram_tensor("wqbT", [Q_LORA, HPC * QK_HD], bf,
                          kind="ExternalInput")
    wkvbT = nc.dram_tensor("wkvbT", [KV_LORA, HPC * (NOPE + V_DIM)], bf,
                           kind="ExternalInput")
    # wo.T rows regrouped head-major: rows [h*512 + g'*128 ...] = head g'*4+h
    woTr = nc.dram_tensor("woTr", [N_HEADS * V_DIM, DIM], bf,
                          kind="ExternalInput")
    cos_sh = nc.dram_tensor("cos_sh", [P, SSH], bf, kind="ExternalInput")
    sin_sh = nc.dram_tensor("sin_sh", [P, SSH], bf, kind="ExternalInput")
    cos_full = nc.dram_tensor("cos_full", [P, S], bf, kind="ExternalInput")
    sin_full = nc.dram_tensor("sin_full", [P, S], bf, kind="ExternalInput")
    perm64 = nc.dram_tensor("perm64", [P, P], bf, kind="ExternalInput")
    trimask = nc.dram_tensor("trimask", [P, P], f32, kind="ExternalInput")
    cfg = nc.dram_tensor("cfg", [1, 6], i32, kind="ExternalInput")
    outT = nc.dram_tensor("out", [DIM, SSH], f32, kind="ExternalOutput")

    # ---- internal dram ----
    b_kv = nc.dram_tensor("b_kv", [KVROWS, SSH], bf)
    g_kv = nc.dram_tensor("g_kv", [NCORES * KVROWS, SSH], bf,
                          addr_space="Shared")
    b_q1 = nc.dram_tensor("b_q1", [Q1ROWS, SSH], bf)
    g_q1 = nc.dram_tensor("g_q1", [NCORES * Q1ROWS, SSH], bf,
                          addr_space="Shared")
    b_q2 = nc.dram_tensor("b_q2", [Q2ROWS, SSH], bf)
    g_q2 = nc.dram_tensor("g_q2", [NCORES * Q2ROWS, SSH], bf,
                          addr_space="Shared")
    o_bnc = [nc.dram_tensor(f"o_bnc{h}", [V_DIM, S], bf) for h in range(HPC)]
    o_g = [nc.dram_tensor(f"o_g{h}", [NCORES * V_DIM, S], bf,
                          ) for h in range(HPC)]
    rg = [list(range(NCORES))]

    with tile.TileContext(nc) as tc:
        with (
            tc.tile_pool(name="persist", bufs=1) as persist,
            tc.tile_pool(name="attn", bufs=1) as attn_pool,
            tc.tile_pool(name="wts", bufs=1) as wts,
        ):
            # constants (cheap, engine-local)
            ones_f = persist.tile([P, 1], f32)
            nc.vector.memset(ones_f, 1.0)
            ones_b = persist.tile([P, 1], bf)
            nc.vector.memset(ones_b, 1.0)
            eps_sb = persist.tile([1, 1], f32)
            nc.vector.memset(eps_sb, EPS)
            cfg_sb = persist.tile([1, 6], i32)
            nc.sync.dma_start(out=cfg_sb, in_=cfg[:])

            # per-core dynamic offsets
            r0 = nc.alloc_registers()
            nc.regs_load(r0, cfg_sb[0:1, 0:1])
            kv_base = nc.snap(r0, donate=True, min_val=0,
                              max_val=GROUP * KVROWS)
            r1 = nc.alloc_registers()
            nc.regs_load(r1, cfg_sb[0:1, 1:2])
            q1_base = nc.snap(r1, donate=True, min_val=0,
                              max_val=GROUP * Q1ROWS)
            r2 = nc.alloc_registers()
            nc.regs_load(r2, cfg_sb[0:1, 2:3])
            q2_base = nc.snap(r2, donate=True, min_val=0,
                              max_val=GROUP * Q2ROWS)
            r3 = nc.alloc_registers()
            nc.regs_load(r3, cfg_sb[0:1, 3:4])
            og_col = nc.snap(r3, donate=True, min_val=0, max_val=S - 512)
            r4 = nc.alloc_registers()
            nc.regs_load(r4, cfg_sb[0:1, 4:5])
            og_row = nc.snap(r4, donate=True, min_val=0, max_val=S // 4)

            # attention-phase persistent tiles (filled by up-proj)
            qt_nope = [attn_pool.tile([P, S], bf, tag=f"qtn{h}",
                                      name=f"qt_nope{h}") for h in range(HPC)]
            qt_pe = [attn_pool.tile([P, S], bf, tag=f"qtp{h}",
                                    name=f"qt_pe{h}")
                     for h in range(HPC // 2)]
            kt_nope = [attn_pool.tile([P, S], bf, tag=f"ktn{h}",
                                      name=f"kt_nope{h}") for h in range(HPC)]
            v_all = attn_pool.tile([P, S // P, HPC * V_DIM], bf)

            # weights/tables prefetched on the scalar SWDGE queue (idle
            # early) so they don't delay phase-A HWDGE traffic
            perm_sb = persist.tile([P, P], bf)
            nc.sync.dma_start(out=perm_sb, in_=perm64[:])
            cos_sh_sb = persist.tile([P, SSH], bf)
            nc.sync.dma_start(out=cos_sh_sb, in_=cos_sh[:])
            sin_sh_sb = persist.tile([P, SSH], bf)
            nc.sync.dma_start(out=sin_sh_sb, in_=sin_sh[:])
            wkvb = wts.tile([P, NKVM, HPC * (NOPE + V_DIM)], bf)
            wqb = wts.tile([P, NQM, HPC * QK_HD], bf)
            mask_sb = persist.tile([P, P], f32)
            cos_f_sb = persist.tile([P, S], bf)
            sin_f_sb = persist.tile([P, S], bf)

            # ======== Phase A + up-proj (shared latent pool) ========
            up_lat_cm = tc.tile_pool(name="up_lat", bufs=1)
            up_lat = up_lat_cm.__enter__()
            with (
                tc.tile_pool(name="pa", bufs=2) as pa,
                tc.tile_pool(name="pa_x", bufs=1) as pa_x,
                tc.tile_pool(name="pa_out", bufs=3) as pa_out,
                tc.tile_pool(name="pa_ps", bufs=2, space="PSUM") as pa_ps,
                tc.tile_pool(name="pa_st", bufs=1, space="PSUM") as pa_st,
            ):
                x_all = pa_x.tile([P, NKT, SSH], bf)
                for xq in range(4):
                    nc.sync.dma_start(
                        out=x_all[:, xq * 4:(xq + 1) * 4, :],
                        in_=xT[xq * 4 * P:(xq + 1) * 4 * P, :].rearrange(
                            "(kt p) s -> p kt s", p=P))

                q_stat = pa_st.tile([1, SSH], f32)
                kv_stat = pa_st.tile([1, SSH], f32)

                def down_slab(wT, m0, mrows, bounce, dst_rows, stat_ps,
                              stat_first, stat_last, ev_tag="ev"):
                    slab = pa.tile([P, NKT, mrows], bf, tag="slab")
                    nc.sync.dma_start(
                        out=slab,
                        in_=wT[:, m0:m0 + mrows].rearrange(
                            "(kt p) m -> p kt m", p=P))
                    ps = pa_ps.tile([P, SSH], f32, tag="dps")
                    for k in range(NKT):
                        nc.tensor.matmul(ps[:mrows, :], slab[:, k, :],
                                         x_all[:, k, :], start=(k == 0),
                                         stop=(k == NKT - 1))
                    ev = pa_out.tile([P, SSH], bf, tag=ev_tag)
                    nc.vector.tensor_copy(ev[:mrows, :], ps[:mrows, :])
                    if dst_rows is not None:
                        nc.sync.dma_start(
                            out=bounce[dst_rows:dst_rows + mrows, :],
                            in_=ev[:mrows, :])
                    if stat_ps is not None:
                        sq = pa.tile([P, SSH], f32, tag="sq")
                        nc.scalar.square(sq[:mrows, :], ps[:mrows, :])
                        nc.tensor.matmul(stat_ps, ones_f[:mrows, :],
                                         sq[:mrows, :], start=stat_first,
                                         stop=stat_last)
                    return ev

                def stat_row(stat, n, bounce, row):
                    tmp = pa.tile([1, SSH], f32, tag="srt")
                    nc.scalar.activation(tmp, stat,
                                         mybir.ActivationFunctionType.Sqrt,
                                         bias=eps_sb[0:1, 0:1], scale=1.0 / n)
                    rcp = pa.tile([1, SSH], f32, tag="rcp")
                    nc.vector.reciprocal(rcp, tmp)
                    rb = pa.tile([1, SSH], bf, tag="rb")
                    nc.vector.tensor_copy(rb, rcp)
                    nc.sync.dma_start(out=bounce[row:row + 1, :], in_=rb)

                # ---- kv first (so AG_kv overlaps q down-proj) ----
                for m in range(NKVM):
                    down_slab(wkvaT, m * P, P, b_kv, m * P, kv_stat,
                              m == 0, m == NKVM - 1)
                kpe_ev = down_slab(wkvaT, KV_LORA, ROPE, None, None, None,
                                   False, False, ev_tag="kpe_ev")
                xs_ps = pa_ps.tile([ROPE, SSH], f32, tag="xs")
                nc.tensor.matmul(xs_ps, perm_sb[:ROPE, :ROPE], kpe_ev[:ROPE, :])
                y0 = pa.tile([ROPE, SSH], bf, tag="ry0")
                nc.vector.tensor_mul(y0, kpe_ev[:ROPE, :], cos_sh_sb[:ROPE, :])
                y1 = pa.tile([ROPE, SSH], bf, tag="ry1")
                nc.vector.tensor_mul(y1, xs_ps, sin_sh_sb[:ROPE, :])
                yr = pa.tile([ROPE, SSH], bf, tag="ryr")
                nc.vector.tensor_add(yr, y0, y1)
                nc.sync.dma_start(out=b_kv[KV_LORA:KV_LORA + ROPE, :], in_=yr)
                stat_row(kv_stat, KV_LORA, b_kv, KVROWS - 1)

                nc.gpsimd.collective_compute(
                    "AllGather", mybir.AluOpType.bypass, replica_groups=rg,
                    ins=[b_kv[:]], outs=[g_kv[:]])

                # weight/table prefetch on scalar SWDGE (idle after the
                # phase-A stat chain; needed only from the up-proj on)
                nc.scalar.dma_start(
                    out=wkvb,
                    in_=wkvbT[:].rearrange("(kt p) m -> p kt m", p=P))
                nc.scalar.dma_start(
                    out=wqb, in_=wqbT[:].rearrange("(kt p) m -> p kt m", p=P))
                nc.scalar.dma_start(out=mask_sb, in_=trimask[:])
                nc.scalar.dma_start(out=cos_f_sb, in_=cos_full[:])
                nc.scalar.dma_start(out=sin_f_sb, in_=sin_full[:])

                # ---- q down-proj (overlaps AG_kv / AG_q1) ----
                for m in range(NQ1):
                    down_slab(wqaT, m * P, P, b_q1, m * P, q_stat,
                              m == 0, False)
                nc.gpsimd.collective_compute(
                    "AllGather", mybir.AluOpType.bypass, replica_groups=rg,
                    ins=[b_q1[:]], outs=[g_q1[:]])

                # kv gather loads: on sync HWDGE here — AG_kv is done by
                # now, so they don't block the remaining q2 slab DMAs
                kv_lat = up_lat.tile([P, NKVM, NCH, 512], bf)
                akv_row = up_lat.tile([1, NCH, 512], bf)
                kpe_dup = attn_pool.tile([P, NCH, 512], bf)
                for r in range(NCH):
                    nc.sync.dma_start(
                        out=akv_row[0:1, r, :],
                        in_=g_kv[bass.ds(kv_base + r * KVROWS + KVROWS - 1,
                                         1), :])
                a_kv_bc = up_lat.tile([P, NCH, 512], bf)
                for r in range(NCH):
                    nc.gpsimd.partition_broadcast(a_kv_bc[:, r, :],
                                                  akv_row[0:1, r, :])
                for r in range(NCH):
                    nc.sync.dma_start(
                        out=kpe_dup[:ROPE, r, :],
                        in_=g_kv[bass.ds(kv_base + r * KVROWS + KV_LORA,
                                         ROPE), :])
                    nc.sync.dma_start(
                        out=kpe_dup[ROPE:, r, :],
                        in_=g_kv[bass.ds(kv_base + r * KVROWS + KV_LORA,
                                         ROPE), :])
                    nc.sync.dma_start(
                        out=kv_lat[:, :, r, :],
                        in_=g_kv[bass.ds(kv_base + r * KVROWS, KV_LORA), :]
                        .rearrange("(kt p) s -> p kt s", p=P))

                for m in range(NQ1, NQM):
                    down_slab(wqaT, m * P, P, b_q2, (m - NQ1) * P, q_stat,
                              False, m == NQM - 1)
                stat_row(q_stat, Q_LORA, b_q2, Q2ROWS - 1)
                nc.gpsimd.collective_compute(
                    "AllGather", mybir.AluOpType.bypass, replica_groups=rg,
                    ins=[b_q2[:]], outs=[g_q2[:]])

                # a_q rows (q_lat itself is streamed per-chunk in q-up)
                aq_row = up_lat.tile([1, NCH, 512], bf)
                for r in range(NCH):
                    nc.gpsimd.dma_start(
                        out=aq_row[0:1, r, :],
                        in_=g_q2[bass.ds(q2_base + r * Q2ROWS + Q2ROWS - 1,
                                         1), :])
                a_q_bc = up_lat.tile([P, NCH, 512], bf)
                for r in range(NCH):
                    nc.gpsimd.partition_broadcast(a_q_bc[:, r, :],
                                                  aq_row[0:1, r, :])

            # ================= Phase B: up projections =================
            with (
                tc.tile_pool(name="up", bufs=3) as up,
                tc.tile_pool(name="qlat", bufs=2) as qlat_pool,
                tc.tile_pool(name="up_ps", bufs=3, space="PSUM") as up_ps,
                tc.tile_pool(name="pe_ps", bufs=2, space="PSUM") as pe_ps,
            ):
                # prescale kv latent by inv-rms (q scaled at eviction)
                for k in range(NKVM):
                    for r in range(NCH):
                        nc.vector.tensor_mul(kv_lat[:, k, r, :],
                                             kv_lat[:, k, r, :],
                                             a_kv_bc[:, r, :])
                # k_nope (d-major) + v (row-major)
                for c in range(NCH):
                    for h in range(HPC):
                        ps = up_ps.tile([P, 512], f32, tag="up")
                        for k in range(NKVM):
                            nc.tensor.matmul(
                                ps, wkvb[:, k, h * NOPE:(h + 1) * NOPE],
                                kv_lat[:, k, c, :], start=(k == 0),
                                stop=(k == NKVM - 1))
                        nc.vector.tensor_copy(
                            kt_nope[h][:, c * 512:(c + 1) * 512], ps)
                for sb in range(S // P):
                    c, part = sb // 4, sb % 4
                    ps = up_ps.tile([P, HPC * V_DIM], f32, tag="up")
                    for k in range(NKVM):
                        nc.tensor.matmul(
                            ps, kv_lat[:, k, c, part * P:(part + 1) * P],
                            wkvb[:, k, HPC * NOPE:], start=(k == 0),
                            stop=(k == NKVM - 1))
                    nc.vector.tensor_copy(v_all[:, sb, :], ps)

                # ---- q up-proj (waits on AG_q1/2; q_lat streamed) ----
                for c in range(NCH):
                    ql = qlat_pool.tile([P, NQM, 512], bf, tag="ql",
                                        name="ql")
                    nc.sync.dma_start(
                        out=ql[:, 0:NQ1, :],
                        in_=g_q1[bass.ds(q1_base + c * Q1ROWS, Q1ROWS), :]
                        .rearrange("(kt p) s -> p kt s", p=P))
                    nc.sync.dma_start(
                        out=ql[:, NQ1:, :],
                        in_=g_q2[bass.ds(q2_base + c * Q2ROWS, Q2ROWS - 1), :]
                        .rearrange("(kt p) s -> p kt s", p=P))
                    for h in range(HPC):
                        ps = up_ps.tile([P, 512], f32, tag="up")
                        for k in range(NQM):
                            nc.tensor.matmul(
                                ps, wqb[:, k, h * P:(h + 1) * P],
                                ql[:, k, :], start=(k == 0),
                                stop=(k == NQM - 1))
                        nc.vector.tensor_mul(
                            qt_nope[h][:, c * 512:(c + 1) * 512], ps,
                            a_q_bc[:, c, :])
                    for hp in range(HPC // 2):
                        pcol0 = HPC * NOPE + 2 * hp * ROPE
                        pcol1 = pcol0 + ROPE
                        ps = pe_ps.tile([P, 512], f32, tag="qp")
                        for k in range(NQM):
                            nc.tensor.matmul(
                                ps[0:ROPE, :], wqb[:, k, pcol0:pcol0 + ROPE],
                                ql[:, k, :], start=(k == 0),
                                stop=(k == NQM - 1), tile_position=(0, 0))
                            nc.tensor.matmul(
                                ps[ROPE:, :], wqb[:, k, pcol1:pcol1 + ROPE],
                                ql[:, k, :], start=(k == 0),
                                stop=(k == NQM - 1), tile_position=(0, ROPE))
                        pe_s = up.tile([P, 512], bf, tag="pes")
                        nc.vector.tensor_mul(pe_s, ps, a_q_bc[:, c, :])
                        xs = pe_ps.tile([P, 512], f32, tag="qpx")
                        nc.tensor.matmul(xs, perm_sb, pe_s)
                        dst = qt_pe[hp][:, c * 512:(c + 1) * 512]
                        nc.vector.tensor_mul(
                            dst, pe_s, cos_f_sb[:, c * 512:(c + 1) * 512])
                        t1 = up.tile([P, 512], bf, tag="pet")
                        nc.vector.tensor_mul(
                            t1, xs, sin_f_sb[:, c * 512:(c + 1) * 512])
                        nc.vector.tensor_add(dst, dst, t1)

            up_lat_cm.__exit__(None, None, None)

            # ========== attention + per-head AGs + interleaved wo ==========
            with (
                tc.tile_pool(name="at", bufs=3) as at,
                tc.tile_pool(name="at_rl", bufs=2) as at_rl,
                tc.tile_pool(name="wo_rhs", bufs=2) as wo_rhs,
                tc.tile_pool(name="wo_acc", bufs=1) as wo_acc,
                tc.tile_pool(name="wo_w", bufs=2) as wo_w,
                tc.tile_pool(name="wo_ev", bufs=3) as wo_ev,
                tc.tile_pool(name="st_ps", bufs=2, space="PSUM") as st_ps,
                tc.tile_pool(name="ot_ps", bufs=2, space="PSUM") as ot_ps,
                tc.tile_pool(name="l_ps", bufs=2, space="PSUM") as l_ps,
                tc.tile_pool(name="wo_ps", bufs=2, space="PSUM") as wo_ps,
            ):
                acc = wo_acc.tile([P, NKT, 512], f32)

                def attention_head(h):
                    pending = None  # (pj, off, j, ot, lt, first, last)

                    def flush():
                        nonlocal pending
                        if pending is None:
                            return
                        pj, off, j, ot, lt, first, last = pending
                        nc.tensor.matmul(lt[:, off:], ones_b, pj[:, off:],
                                         start=first, stop=last)
                        nc.tensor.matmul(
                            ot[:, off:],
                            v_all[:, j, h * V_DIM:(h + 1) * V_DIM],
                            pj[:, off:], start=first, stop=last)
                        pending = None

                    evs = []
                    for qc in range(NCH):
                        nj = qc * 4 + 4
                        ot = ot_ps.tile([P, 512], f32, tag="ot", name="ot")
                        lt = l_ps.tile([1, 512], f32, tag="l", name="lt")
                        for j in range(nj):
                            d = j - qc * 4
                            off = max(0, d) * P
                            st = st_ps.tile([P, 512], f32, tag="st",
                                            name="st")
                            nc.tensor.matmul(
                                st[:, off:],
                                kt_nope[h][:, j * P:(j + 1) * P],
                                qt_nope[h][:, qc * 512 + off:(qc + 1) * 512],
                                start=True, stop=False)
                            lo = (h % 2) * ROPE
                            nc.tensor.matmul(
                                st[:, off:],
                                kpe_dup[lo:lo + ROPE, j // 4,
                                        (j % 4) * P:(j % 4 + 1) * P],
                                qt_pe[h // 2][lo:lo + ROPE,
                                              qc * 512 + off:(qc + 1) * 512],
                                start=False, stop=True)
                            flush()
                            if d >= 0:
                                nc.vector.tensor_add(st[:, off:off + P],
                                                     st[:, off:off + P],
                                                     mask_sb)
                            pj = at.tile([P, 512], bf, tag="p", name="pj")
                            nc.scalar.activation(
                                pj[:, off:], st[:, off:],
                                mybir.ActivationFunctionType.Exp)
                            pending = (pj, off, j, ot, lt, j == 0,
                                       j == nj - 1)
                        # evictions of previous qc happen via evs below
                        evs.append((ot, lt, qc))
                    flush()
                    for ot, lt, qc in evs:
                        rl = at_rl.tile([1, 512], f32, tag="rl", name="rl")
                        nc.vector.reciprocal(rl, lt)
                        rlb = at_rl.tile([P, 512], f32, tag="rlb",
                                         name="rlb")
                        nc.gpsimd.partition_broadcast(rlb, rl)
                        ev = at.tile([P, 512], bf, tag="oev", name="oev")
                        nc.vector.tensor_mul(ev, ot, rlb)
                        nc.sync.dma_start(
                            out=o_bnc[h][:, qc * 512:(qc + 1) * 512],
                            in_=ev)
                    nc.gpsimd.collective_compute(
                        "AllGather", mybir.AluOpType.bypass,
                        replica_groups=rg, ins=[o_bnc[h][:]],
                        outs=[o_g[h][:]])
                    # rhs loads for the wo pass of this head (gpsimd queue,
                    # blocked only by this AG)
                    rhs = wo_rhs.tile([P, GROUP, 512], bf, tag="rhs",
                                      name="rhs")
                    for k in range(GROUP):
                        nc.gpsimd.dma_start(
                            out=rhs[:, k, :],
                            in_=o_g[h][bass.ds(og_row + k * P, P),
                                       bass.ds(og_col, 512)])
                    # prefetch this head's wo weights (no deps)
                    wslab = wo_w.tile([P, GROUP, DIM], bf, tag="woslab",
                                      name="wslab")
                    nc.scalar.dma_start(
                        out=wslab,
                        in_=woTr[h * 512:(h + 1) * 512, :].rearrange(
                            "(kt p) m -> p kt m", p=P))
                    return rhs, wslab

                def wo_pass(h, rhs, wslab):
                    for m in range(NKT):
                        ps = wo_ps.tile([P, 512], f32, tag="wops",
                                        name="wops")
                        for k in range(GROUP):
                            nc.tensor.matmul(
                                ps, wslab[:, k, m * P:(m + 1) * P],
                                rhs[:, k, :], start=(k == 0),
                                stop=(k == GROUP - 1))
                        if h == 0:
                            nc.vector.tensor_copy(acc[:, m, :], ps)
                        elif h < HPC - 1:
                            nc.vector.tensor_add(acc[:, m, :], ps,
                                                 acc[:, m, :])
                        else:
                            ev = wo_ev.tile([P, 512], f32, tag="woev",
                                            name="woev")
                            nc.vector.tensor_add(ev, ps, acc[:, m, :])
                            nc.sync.dma_start(out=outT[m * P:(m + 1) * P, :],
                                              in_=ev)

                heads_rhs = {}
                for h in range(HPC):
                    heads_rhs[h] = attention_head(h)
                    if h >= 1:
                        wo_pass(h - 1, *heads_rhs[h - 1])
                wo_pass(HPC - 1, *heads_rhs[HPC - 1])

    nc.compile()
    return nc


def _prep_inputs(x, freqs_cos, freqs_sin, wq_a, q_norm_w, wq_b, wkv_a,
                 kv_norm_w, wkv_b, wo):
    x = np.asarray(x, np.float32)
    freqs_cos = np.asarray(freqs_cos, np.float32)
    freqs_sin = np.asarray(freqs_sin, np.float32)
    wq_a = np.asarray(wq_a, np.float32)
    q_norm_w = np.asarray(q_norm_w, np.float32)
    wq_b = np.asarray(wq_b, np.float32)
    wkv_a = np.asarray(wkv_a, np.float32)
    kv_norm_w = np.asarray(kv_norm_w, np.float32)
    wkv_b = np.asarray(wkv_b, np.float32)
    wo = np.asarray(wo, np.float32)

    wqaT = np.ascontiguousarray(wq_a.T).astype(BF16)
    wkvaT = np.ascontiguousarray(wkv_a.T).astype(BF16)

    wqb_eff = (wq_b * q_norm_w[None, :]) * SCALE
    wqb_eff = wqb_eff.reshape(N_HEADS, QK_HD, Q_LORA)
    wkvb_eff = wkv_b * kv_norm_w[None, :]
    wkvb_eff = wkvb_eff.reshape(N_HEADS, NOPE + V_DIM, KV_LORA)

    cosT = np.tile(np.repeat(freqs_cos.T, 2, axis=0), (2, 1))  # [128, S]
    sinT = np.tile(np.repeat(freqs_sin.T, 2, axis=0), (2, 1))

    perm64_ = np.zeros((ROPE, ROPE), np.float32)
    for i in range(ROPE // 2):
        perm64_[2 * i + 1, 2 * i] = -1.0  # out[2i]   = -x[2i+1]
        perm64_[2 * i, 2 * i + 1] = 1.0   # out[2i+1] =  x[2i]
    perm = np.zeros((P, P), np.float32)
    perm[:ROPE, :ROPE] = perm64_
    perm[ROPE:, ROPE:] = perm64_
    r = np.arange(P)
    trimask = np.where(r[:, None] <= r[None, :], 0.0,
                       -1e30).astype(np.float32)

    # wo.T rows regrouped so pass h contracts head g'*4+h for g'=0..3:
    # woTr rows [h*512 + g'*128 : ...] = wo.T rows of head g'*4+h
    woT4 = wo.T.reshape(N_HEADS // 4, 4, V_DIM, DIM)  # [g', h, 128, D]
    woTr = np.ascontiguousarray(
        woT4.transpose(1, 0, 2, 3).reshape(N_HEADS * V_DIM, DIM)).astype(BF16)

    in_maps = []
    for c in range(NCORES):
        b, g = c // GROUP, c % GROUP
        heads = slice(g * HPC, (g + 1) * HPC)
        xTc = np.ascontiguousarray(
            x[b].T[:, g * SSH:(g + 1) * SSH]).astype(BF16)
        wqbT = np.concatenate(
            [wqb_eff[heads, :NOPE].reshape(HPC * NOPE, Q_LORA),
             wqb_eff[heads, NOPE:].reshape(HPC * ROPE, Q_LORA)],
            axis=0).T
        wkvbT = np.concatenate(
            [wkvb_eff[heads, :NOPE].reshape(HPC * NOPE, KV_LORA),
             wkvb_eff[heads, NOPE:].reshape(HPC * V_DIM, KV_LORA)],
            axis=0).T
        in_maps.append({
            "xT": xTc,
            "wqaT": wqaT,
            "wkvaT": wkvaT,
            "wqbT": np.ascontiguousarray(wqbT).astype(BF16),
            "wkvbT": np.ascontiguousarray(wkvbT).astype(BF16),
            "woTr": woTr,
            "cos_sh": np.ascontiguousarray(
                cosT[:, g * SSH:(g + 1) * SSH]).astype(BF16),
            "sin_sh": np.ascontiguousarray(
                sinT[:, g * SSH:(g + 1) * SSH]).astype(BF16),
            "cos_full": np.ascontiguousarray(cosT).astype(BF16),
            "sin_full": np.ascontiguousarray(sinT).astype(BF16),
            "perm64": perm.astype(BF16),
            "trimask": trimask,
            "cfg": np.array([[b * GROUP * KVROWS, b * GROUP * Q1ROWS,
                              b * GROUP * Q2ROWS, g * 512, b * 512, 0]],
                            np.int32),
        })
    return in_maps


def _run(inputs, trace=False, **kw):
    if "nc" not in _cache:
        _cache["nc"] = _build()
    nc = _cache["nc"]
    in_maps = _prep_inputs(**inputs)
    res = run_bass_kernel_spmd(nc, in_maps, core_ids=list(range(NCORES)),
                               trace=trace, **kw)
    out = np.empty((B, S, DIM), np.float32)
    for c in range(NCORES):
        b, g = c // GROUP, c % GROUP
        out[b, g * SSH:(g + 1) * SSH, :] = res.results[c]["out"].T
    return out, res


def kernel(**inputs):
    out, _ = _run(inputs)
    return out

